# revision 34
# baseline (speedup 1.0000x reference)
"""Trainium2 Bass kernel for nn_LocalAggregator (GNN message passing).

Math (per batch):
    e[i,j,r] = lrelu( h_i . diag(a_r) . h_j  +  g_r(A_ij) ),
               g_r(a) = sum_t cos(a f_t + p_t) iw[t,r]
    s[i,j]   = e[i,j,adj_ij-1]  if 1<=adj<=5 else -9e15
    out      = softmax_j(s) @ h

Device strategy (per core, 4 of the 32 batches):
  * The time-encoding branch is evaluated ON THE HOST: g is a smooth scalar
    function of A in [0,1); the host fits a degree-6 polynomial per class
    (fit err ~1e-5), evaluates it with the per-element class already
    SELECTED (coefficient gather by adj), folds the adj==0 -> -9e15 mask in,
    and ships one f32 plane GT[j,(b,i)] per core.  This removes the entire
    per-class polynomial pipeline (35 DVE ops in the old kernel) from HW.
  * Scores are computed TRANSPOSED, sT[j,(b,i)]: e1_c = H diag(a_c) H^T is
    symmetric, so the same matmuls serve, and the softmax backend needs no
    PE transposes: exp(sT) is directly the matmul lhsT for out = alpha @ h;
    row sums come from a ones-vector matmul sharing the same stationary.
  * e1 matmuls run in fp16 (1 PE cycle/row vs 4 for fp32) with the
    UNSCALED hT chunk as the stationary, shared by all 5 classes: per
    (batch, k-chunk) one weight load + 2 matmuls into a class-stacked PSUM
    layout (classes 0-3 in one 512-wide bank per batch, class 4 packed
    (b,j) in a fifth bank).
  * hTa = hT * a_c scaling runs on-chip in fp16 tensor_scalar ops (DVE,
    two tail classes on the Act engine), filling the input-DMA shadow.
  * Class select happens IN PLACE in PSUM: 4 full-width DVE copy_predicated
    (host-built int8 masks) overwrite the class-0 bank slots; then DVE
    computes s = lrelu(s + GT) into SBUF (adj==0 rides on GT=-9e15).
    Custom fused DVE ops are unusable (walrus "ISA wrong length").
  * Fixed-cost hygiene: PE warmup matmuls hold the DVFS clock up through
    the DMA wait; the scalar engine's Exp table is prewarmed (Lrelu lives
    in a different act table, so lrelu stays on DVE); input DMAs are
    split across the three DMA rings with batch-0 quarters first.
  * Softmax needs no max-shift (scores bounded ~35, exp fits f32/bf16
    range); exp emits bf16 so the backend matmuls run at 1 cycle/row.
  * Two walrus version-skew workarounds retained from the baseline: the
    Tile tail drain and the one-sync-wait-per-instruction limit
    (_patch_tail_drain / _split_excess_waits).
"""

import os
from contextlib import ExitStack

import numpy as np
import ml_dtypes

B, N, D, TDIM = 32, 128, 256, 64
NCORES = 8
BL = B // NCORES            # batches per core
ALPHA = 0.2
NEG_INF = -9e15
DEG = 6                     # host-fitted polynomial degree
DCH = D // 128              # K-chunks for the e1 contraction
FBJ = BL * N                # 512
FBD = BL * D                # 1024

_PROG_CACHE: dict = {}
_DRAIN_PATCHED = False
_WALRUS_PATCHED = False


def _patch_walrus_max_sems(max_sems: int = 64):
    """The walrus NEFF epilogue resets the ENTIRE 256-entry semaphore file,
    one EVENT_SEMAPHORE per sem per engine (~250 ops, ~8us of measured
    teardown). Capping --max-sem-num shrinks the reset to the sems actually
    allocatable."""
    global _WALRUS_PATCHED
    if _WALRUS_PATCHED:
        return
    import concourse.bass_utils as bu

    orig = bu.run_command

    def _patched(cmd, **kw):
        if cmd and str(cmd[0]).endswith("walrus_driver"):
            cmd = list(cmd) + ["--enable-remote-semaphore-dma"]
        return orig(cmd, **kw)

    bu.run_command = _patched
    _WALRUS_PATCHED = True


def _patch_tail_drain():
    """Version-skew workaround: the TileContext tail drain accumulates one
    sem-wait per outstanding engine/DMA queue, but this walrus build's Drain
    encoding fits only ONE sync-wait command. Spread the excess waits over
    preceding single-wait NoOps on the same (SP) engine."""
    global _DRAIN_PATCHED
    if _DRAIN_PATCHED:
        return
    import concourse.tile as tile_mod

    def _patched(self, tick_clock, wait_clock):
        nc = self.nc
        drain_inst = nc.sync.drain()
        wait_clock.add_sem_waits(
            drain_inst.ins,
            tile_mod.ScopedClock({None: tick_clock.global_clock}),
        )
        mi = drain_inst.ins
        si = mi.sync_info
        waits = list(si.on_wait) if si is not None and si.on_wait else []
        if len(waits) > 1:
            si.on_wait = waits[:1]
            lst = nc.cur_bb.bb.instructions
            assert lst[-1] is mi, "drain is not the last instruction in block"
            drain_obj = lst.pop()
            for w in waits[1:]:
                nop = nc.sync.nop(nofuse=True)
                nsi = nop.ins.sync_info
                if nsi is None:
                    nop.ins.sync_info = type(si)(on_update=[], on_wait=[w])
                else:
                    nsi.on_wait = [w]
            lst.append(drain_obj)
        nc.all_engine_barrier()
        assert self.sems is not None
        popped = nc._tile_sem_poison_stack.pop()
        assert popped is self._sem_poison
        # No in-program semaphore clears / second barrier: the runtime's
        # NEFF epilogue re-zeros the whole semaphore file after every
        # execution, so these ops (~1.4us measured) are redundant.

    tile_mod.TileContext._drain_and_barrier = _patched
    _DRAIN_PATCHED = True


def _split_excess_waits(nc, max_waits: int = 1):
    """This walrus build encodes at most one sync-wait command per
    instruction. Hoist excess waits onto same-engine NoOps inserted
    immediately before the over-subscribed instruction."""
    import concourse.mybir as mybir

    for fn in nc.m.functions:
        for bb in fn.blocks:
            insts = bb.instructions
            i = 0
            while i < len(insts):
                inst = insts[i]
                si = getattr(inst, "sync_info", None)
                waits = list(si.on_wait) if si is not None and si.on_wait else []
                if len(waits) > max_waits:
                    si.on_wait = waits[:max_waits]
                    extra = waits[max_waits:]
                    nops = []
                    for k in range(0, len(extra), max_waits):
                        nops.append(
                            mybir.InstNoOp(
                                name=f"{inst.name}-xw{k}",
                                engine=inst.engine,
                                bass_nofuse=True,
                                sync_info=mybir.SyncInfo(
                                    on_wait=extra[k : k + max_waits], on_update=[]
                                ),
                            )
                        )
                    insts[i:i] = nops
                    i += len(nops)
                i += 1


# --------------------------------------------------------------------------
# host-side preprocessing
# --------------------------------------------------------------------------
def _fit_polys(iw_params: np.ndarray, te_freq: np.ndarray, te_phase: np.ndarray):
    """Least-squares fit of g_c(a) = sum_t iw[t,c] cos(a f_t + p_t), a in [0,1].

    Returns C[k, c] for k=0..DEG (monomial basis, increasing order), float64.
    """
    npts = 1024
    x = 0.5 * (1.0 + np.cos(np.pi * (np.arange(npts) + 0.5) / npts))
    f = te_freq.astype(np.float64)
    p = te_phase.astype(np.float64)
    iw = iw_params.astype(np.float64)
    G = np.cos(x[:, None] * f[None, :] + p[None, :]) @ iw      # (npts, 5)
    V = np.vander(x, DEG + 1, increasing=True)                 # (npts, DEG+1)
    C, *_ = np.linalg.lstsq(V, G, rcond=None)
    return C  # (DEG+1, 5) float64


def _host_g_plane(A, adj, Cpoly):
    """Selected time-encoding plane g_{adj}(A), adj==0 -> NEG_INF. (B,N,N) f32."""
    idx = np.clip(adj - 1, 0, 4)
    Af = A.astype(np.float64)
    g = Cpoly[DEG][idx]
    for k in range(DEG - 1, -1, -1):
        g = g * Af + Cpoly[k][idx]
    valid = (adj >= 1) & (adj <= 5)
    return np.where(valid, g, NEG_INF).astype(np.float32)


# --------------------------------------------------------------------------
# Bass program (input-independent; SPMD across 8 cores)
# --------------------------------------------------------------------------
def _build_program():
    import concourse.bass as bass
    import concourse.mybir as mybir
    import concourse.tile as tile

    _patch_tail_drain()

    f32 = mybir.dt.float32
    f16 = mybir.dt.float16
    bf16 = mybir.dt.bfloat16
    i8 = mybir.dt.int8
    Alu = mybir.AluOpType
    Act = mybir.ActivationFunctionType

    nc = bass.Bass()

    # DRAM I/O (per-core layouts; host arranges)
    hT_d = nc.dram_tensor("hT", [128, DCH * BL * 128], f16, kind="ExternalInput")  # [dl,(ch,b,i)]
    a_d = nc.dram_tensor("ap", [128, DCH * 5], f32, kind="ExternalInput")          # [dl,(ch,c)]
    h_d = nc.dram_tensor("h", [N, FBD], bf16, kind="ExternalInput")                # [node,(b,d)]
    gt_d = nc.dram_tensor("gt", [N, FBJ], f32, kind="ExternalInput")               # [j,(b,i)]
    madj_d = nc.dram_tensor("madj", [N, 4 * FBJ], i8, kind="ExternalInput")        # adj==2..5
    out_d = nc.dram_tensor("out", [N, FBD], bf16, kind="ExternalOutput")           # [i,(b,d)]

    with tile.TileContext(nc) as tc, ExitStack() as ctx:
        io = ctx.enter_context(tc.tile_pool(name="io", bufs=1))
        wrk = ctx.enter_context(tc.tile_pool(name="wrk", bufs=1))

        # ---- loads. hT quarters split over the sync+scalar rings, paired
        # so batches 0/1 (ch0 AND ch1) land first and batch-0 e1 can
        # complete earliest. gt/madj/h ride the gpsimd ring (mid/late use).
        hT_sb = io.tile([128, DCH * BL * 128], f16, tag="hT")
        nc.sync.dma_start(hT_sb[:, 0:256], hT_d[:, 0:256])          # ch0 b01
        nc.scalar.dma_start(hT_sb[:, 512:768], hT_d[:, 512:768])    # ch1 b01
        nc.sync.dma_start(hT_sb[:, 256:512], hT_d[:, 256:512])      # ch0 b23
        nc.scalar.dma_start(hT_sb[:, 768:1024], hT_d[:, 768:1024])  # ch1 b23
        cst = io.tile([128, 512], f16, tag="cst")
        nc.gpsimd.memset(cst[:], 0.0)
        one_sb = io.tile([128, 1], bf16, tag="one")
        nc.gpsimd.memset(one_sb[:], 1.0)
        a_sb = io.tile([128, DCH * 5], f32, tag="ap")
        nc.gpsimd.dma_start(a_sb[:], a_d[:])
        madj_sb = io.tile([N, 4 * FBJ], i8, tag="madj")
        nc.gpsimd.dma_start(madj_sb[:], madj_d[:])
        gt_sb = io.tile([N, FBJ], f32, tag="gt")
        nc.gpsimd.dma_start(gt_sb[:], gt_d[:])
        h_sb = io.tile([N, FBD], bf16, tag="h")
        nc.gpsimd.dma_start(h_sb[:], h_d[:])

        # ---- PSUM layout
        psE = ctx.enter_context(tc.tile_pool(name="psE", bufs=1, space="PSUM"))
        E03 = psE.tile([N, BL * 512], f32, tag="E03", name="E03")   # (b, c0..3, j)
        E4 = psE.tile([N, FBJ], f32, tag="E4", name="E4")           # (b, j)
        zw = psE.tile([N, 512], f32, tag="zw", name="zw")           # z cols 0:4 | warmup 256:288
        psB = ctx.enter_context(tc.tile_pool(name="psB", bufs=2, space="PSUM"))

        sT = wrk.tile([N, FBJ], f32, tag="sT")
        exT = wrk.tile([N, FBJ], bf16, tag="exT")
        rz = wrk.tile([N, BL], f32, tag="rz")
        outT = wrk.tile([N, FBD], bf16, tag="outT")

        # ---- warm the scalar engine's Exp act table during the DMA shadow
        # (the load costs 1.3us; Lrelu lives in a DIFFERENT table, so lrelu
        # must stay off the scalar engine or every switch reloads tables)
        exwarm = wrk.tile([128, 1], bf16, tag="exwarm")
        nc.scalar.activation(exwarm[:], cst[:, 0:1], Act.Exp)

        # ---- PE warmup: dummy matmuls keep the PE continuously busy until
        # the real matmuls arrive so DVFS ramps to full clock (a cold PE
        # runs matmuls ~3x slower). 512-free each ~0.4-0.8us of busy.
        for w in range(10):
            nc.tensor.matmul(zw[0:32, 0:512], cst[:, 0:32], cst[:, 0:512],
                             skip_group_check=True)

        # ---- hTa = hT * a_c on DVE tensor_scalar (fp16 2x, ~420ns/op),
        # one op per (class, chunk) spanning all 4 batches; the two
        # ch1-tail classes run on the scalar engine concurrently.
        # hTa layout [dl,(ch,b,c,j)] keeps matmul moving operands contiguous.
        hTa = wrk.tile([128, 5 * DCH * BL * 128], f16, tag="hTa")
        hTa_v = hTa[:].rearrange("p (ch b c j) -> p ch c b j",
                                 ch=DCH, b=BL, c=5, j=128)
        hT_v = hT_sb[:].rearrange("p (ch b i) -> p ch b i", ch=DCH, b=BL, i=128)
        for ch in range(DCH):
            for c in range(5):
                if ch == 1 and c >= 3:
                    nc.scalar.mul(hTa_v[:, ch, c], hT_v[:, ch],
                                  a_sb[:, ch * 5 + c : ch * 5 + c + 1])
                else:
                    nc.vector.tensor_scalar(
                        hTa_v[:, ch, c], hT_v[:, ch],
                        a_sb[:, ch * 5 + c : ch * 5 + c + 1], None, Alu.mult)

        # ---- e1 matmuls: stationary = unscaled hT chunk shared by all 5
        # classes of a (ch,b); classes 0-3 into one 512-wide bank per batch,
        # class 4 packed (b,j) in a fifth bank. fp16, accumulated over ch.
        for b in range(BL):
            for ch in range(DCH):
                lhsT = hT_sb[:, (ch * BL + b) * 128 : (ch * BL + b + 1) * 128]
                mov = (ch * BL + b) * 640
                nc.tensor.matmul(
                    E03[:, b * 512 : (b + 1) * 512],
                    lhsT,
                    hTa[:, mov : mov + 512],
                    start=(ch == 0),
                    stop=(ch == DCH - 1),
                )
                nc.tensor.matmul(
                    E4[:, b * 128 : (b + 1) * 128],
                    lhsT,
                    hTa[:, mov + 512 : mov + 640],
                    start=(ch == 0),
                    stop=(ch == DCH - 1),
                )

        # ---- class select IN PLACE in PSUM: the class-0 slots of E03 become
        # the selected scores (no base copy needed); 4 full-width predicated
        # overwrites for classes 1..4. adj==0 cells get drowned by GT=-9e15.
        Ev = E03[:].rearrange("p (b c j) -> p c b j", b=BL, c=4, j=128)
        sel = Ev[:, 0]                                   # [p, b, j] strides (512,1)
        for k in range(3):   # classes 1..3
            nc.vector.copy_predicated(
                sel,
                madj_sb[:, k * FBJ : (k + 1) * FBJ].rearrange(
                    "p (b j) -> p b j", b=BL, j=128),
                Ev[:, k + 1],
            )
        nc.vector.copy_predicated(
            sel,
            madj_sb[:, 3 * FBJ : 4 * FBJ].rearrange("p (b j) -> p b j", b=BL, j=128),
            E4[:].rearrange("p (b j) -> p b j", b=BL, j=128),
        )

        # ---- s = lrelu(s + GT), exp -> bf16. Full-width ops: less total
        # DVE stream than batch-pair splits and a shallower path to the
        # last exp, which gates the tail. Keep the PE warm meanwhile.
        for w in range(4):
            nc.tensor.matmul(zw[0:32, 0:512], cst[:, 0:32], cst[:, 0:512],
                             skip_group_check=True)
        sel_all = Ev[:, 0]                               # [p, 4, 128]
        sT_v = sT[:].rearrange("p (b j) -> p b j", b=BL, j=128)
        nc.vector.scalar_tensor_tensor(
            sT_v, sel_all, 1.0,
            gt_sb[:].rearrange("p (b j) -> p b j", b=BL, j=128),
            Alu.mult, Alu.add,
        )
        nc.vector.scalar_tensor_tensor(
            sT[:], sT[:], ALPHA, sT[:], Alu.mult, Alu.max)
        nc.scalar.activation(exT[:], sT[:], Act.Exp)

        # ---- softmax backend: z via ones-matmul sharing the exT stationary
        # with the output matmul; rescale by 1/z on the way out; outputs on
        # two rings as batch pairs.
        ops = []
        for b in range(BL):
            exT_b = exT[:, b * 128 : (b + 1) * 128]
            nc.tensor.matmul(zw[:, b : b + 1], exT_b, one_sb[:, 0:1])
            op = psB.tile([N, D], f32, tag="outp", name=f"outp{b}")
            nc.tensor.matmul(op[:], exT_b, h_sb[:, b * D : (b + 1) * D])
            ops.append(op)
            nc.vector.reciprocal(rz[:, b : b + 1], zw[:, b : b + 1])
            # DVE is idle here (exp went full-width): split the four rescale
            # muls across DVE/scalar so the last store issues ~1us sooner
            if b % 2 == 0:
                nc.vector.tensor_scalar(
                    outT[:, b * D : (b + 1) * D], ops[b][:],
                    rz[:, b : b + 1], None, Alu.mult)
            else:
                nc.scalar.mul(outT[:, b * D : (b + 1) * D], ops[b][:],
                              rz[:, b : b + 1])
            q = nc.sync if b % 2 == 0 else nc.gpsimd
            q.dma_start(out_d[:, b * D : (b + 1) * D],
                        outT[:, b * D : (b + 1) * D])

    return nc


# --------------------------------------------------------------------------
# host packing
# --------------------------------------------------------------------------
def _prepare_in_maps(inputs):
    hidden = np.ascontiguousarray(inputs["hidden"], dtype=np.float32)   # (B,N,D)
    A = np.ascontiguousarray(inputs["A_interval"], dtype=np.float32)    # (B,N,N)
    adj = np.asarray(inputs["adj"])                                     # (B,N,N) i32
    a_params = np.asarray(inputs["a_params"], dtype=np.float32)         # (D,5)
    iw = np.asarray(inputs["iw_params"])
    f = np.asarray(inputs["te_freq"])
    p = np.asarray(inputs["te_phase"])

    Cpoly = _fit_polys(iw, f, p)
    G = _host_g_plane(A, adj, Cpoly)                                    # (B,N,N) f32

    in_maps = []
    for core in range(NCORES):
        bs = slice(core * BL, (core + 1) * BL)
        hs = hidden[bs]                                   # (BL,N,D)
        assert ((adj[bs] >= 1) & (adj[bs] <= 5)).any(axis=2).all(), (
            "row with no valid edge: shift-free softmax unsupported")
        # h: [node, (b,d)] bf16
        h_host = np.ascontiguousarray(hs.transpose(1, 0, 2)).reshape(
            N, FBD).astype(ml_dtypes.bfloat16)
        # hT: [dl, (ch, b, i)] fp16
        hT_host = np.empty((128, DCH * BL * 128), np.float16)
        for ch in range(DCH):
            for b in range(BL):
                hT_host[:, (ch * BL + b) * 128 : (ch * BL + b + 1) * 128] = (
                    hs[b, :, ch * 128 : (ch + 1) * 128].T
                )
        # a_params -> [dl, (ch, c)] fp16
        ap_host = np.empty((128, DCH * 5), np.float32)
        for ch in range(DCH):
            ap_host[:, ch * 5 : (ch + 1) * 5] = a_params[ch * 128 : (ch + 1) * 128, :]
        # transposed planes [j, (b, i)]
        GT_host = np.ascontiguousarray(G[bs].transpose(2, 0, 1)).reshape(N, FBJ)
        adjT = adj[bs].transpose(2, 0, 1).reshape(N, FBJ)
        madj_host = np.empty((N, 4 * FBJ), np.int8)
        for k in range(4):  # classes 1..4  <->  adj == 2..5
            madj_host[:, k * FBJ : (k + 1) * FBJ] = (adjT == k + 2)
        in_maps.append({
            "hT": hT_host, "ap": ap_host, "h": h_host,
            "gt": GT_host, "madj": madj_host,
        })
    return in_maps


def _get_program():
    nc = _PROG_CACHE.get("prog")
    if nc is None:
        nc = _build_program()
        _split_excess_waits(nc)
        _PROG_CACHE["prog"] = nc
    return nc


# --------------------------------------------------------------------------
# public entry point
# --------------------------------------------------------------------------
def kernel(**inputs: np.ndarray) -> np.ndarray:
    nc = _get_program()
    in_maps = _prepare_in_maps(inputs)

    from concourse.bass_utils import run_bass_kernel_spmd

    res = run_bass_kernel_spmd(nc, in_maps, core_ids=list(range(NCORES)))
    out = np.empty((B, N, D), np.float32)
    for core in range(NCORES):
        o = np.asarray(res.results[core]["out"]).astype(np.float32)
        out[core * BL : (core + 1) * BL] = o.reshape(N, BL, D).transpose(1, 0, 2)
    return out


if __name__ == "__main__":
    rng = np.random.default_rng(0)
    demo = {
        "hidden": rng.standard_normal((B, N, D), dtype=np.float32),
        "A_interval": rng.random((B, N, N), dtype=np.float32),
        "adj": rng.integers(0, 6, (B, N, N)).astype(np.int32),
        "interval_unique": rng.integers(0, 100, (B, N)).astype(np.int32),
        "mask_item": rng.integers(0, 2, (B, N)).astype(np.int32),
        "a_params": (rng.standard_normal((D, 5)) / np.sqrt(D)).astype(np.float32),
        "iw_params": rng.standard_normal((TDIM, 5)).astype(np.float32),
        "te_freq": rng.standard_normal(TDIM).astype(np.float32),
        "te_phase": rng.standard_normal(TDIM).astype(np.float32),
    }
    o = kernel(**demo)
    print("kernel output", o.shape, o.dtype, np.abs(o).max())


# revision 35
# speedup vs baseline: 1.0397x; 1.0397x over previous
"""Trainium2 Bass kernel for nn_LocalAggregator (GNN message passing).

Math (per batch):
    e[i,j,r] = lrelu( h_i . diag(a_r) . h_j  +  g_r(A_ij) ),
               g_r(a) = sum_t cos(a f_t + p_t) iw[t,r]
    s[i,j]   = e[i,j,adj_ij-1]  if 1<=adj<=5 else -9e15
    out      = softmax_j(s) @ h

Device strategy (per core, 4 of the 32 batches):
  * The time-encoding branch is evaluated ON THE HOST: g is a smooth scalar
    function of A in [0,1); the host fits a degree-6 polynomial per class
    (fit err ~1e-5), evaluates it with the per-element class already
    SELECTED (coefficient gather by adj), folds the adj==0 -> -9e15 mask in,
    and ships one f32 plane GT[j,(b,i)] per core.  This removes the entire
    per-class polynomial pipeline (35 DVE ops in the old kernel) from HW.
  * Scores are computed TRANSPOSED, sT[j,(b,i)]: e1_c = H diag(a_c) H^T is
    symmetric, so the same matmuls serve, and the softmax backend needs no
    PE transposes: exp(sT) is directly the matmul lhsT for out = alpha @ h;
    row sums come from a ones-vector matmul sharing the same stationary.
  * e1 matmuls run in fp16 (1 PE cycle/row vs 4 for fp32) with the
    UNSCALED hT chunk as the stationary, shared by all 5 classes: per
    (batch, k-chunk) one weight load + 2 matmuls into a class-stacked PSUM
    layout (classes 0-3 in one 512-wide bank per batch, class 4 packed
    (b,j) in a fifth bank).
  * hTa = hT * a_c scaling runs on-chip in fp16 tensor_scalar ops (DVE,
    two tail classes on the Act engine), filling the input-DMA shadow.
  * Class select happens IN PLACE in PSUM: 4 full-width DVE copy_predicated
    (host-built int8 masks) overwrite the class-0 bank slots; then DVE
    computes s = lrelu(s + GT) into SBUF (adj==0 rides on GT=-9e15).
    Custom fused DVE ops are unusable (walrus "ISA wrong length").
  * Fixed-cost hygiene: PE warmup matmuls hold the DVFS clock up through
    the DMA wait; the scalar engine's Exp table is prewarmed (Lrelu lives
    in a different act table, so lrelu stays on DVE); input DMAs are
    split across the three DMA rings with batch-0 quarters first.
  * Softmax needs no max-shift (scores bounded ~35, exp fits f32/bf16
    range); exp emits bf16 so the backend matmuls run at 1 cycle/row.
  * Two walrus version-skew workarounds retained from the baseline: the
    Tile tail drain and the one-sync-wait-per-instruction limit
    (_patch_tail_drain / _split_excess_waits).
"""

import os
from contextlib import ExitStack

import numpy as np
import ml_dtypes

B, N, D, TDIM = 32, 128, 256, 64
NCORES = 8
BL = B // NCORES            # batches per core
ALPHA = 0.2
NEG_INF = -9e15
DEG = 6                     # host-fitted polynomial degree
DCH = D // 128              # K-chunks for the e1 contraction
FBJ = BL * N                # 512
FBD = BL * D                # 1024

_PROG_CACHE: dict = {}
_DRAIN_PATCHED = False
_WALRUS_PATCHED = False


def _patch_walrus_max_sems(max_sems: int = 64):
    """The walrus NEFF epilogue resets the ENTIRE 256-entry semaphore file,
    one EVENT_SEMAPHORE per sem per engine (~250 ops, ~8us of measured
    teardown). Capping --max-sem-num shrinks the reset to the sems actually
    allocatable."""
    global _WALRUS_PATCHED
    if _WALRUS_PATCHED:
        return
    import concourse.bass_utils as bu

    orig = bu.run_command

    def _patched(cmd, **kw):
        if cmd and str(cmd[0]).endswith("walrus_driver"):
            cmd = list(cmd) + ["--enable-remote-semaphore-dma"]
        return orig(cmd, **kw)

    bu.run_command = _patched
    _WALRUS_PATCHED = True


def _patch_tail_drain():
    """Version-skew workaround: the TileContext tail drain accumulates one
    sem-wait per outstanding engine/DMA queue, but this walrus build's Drain
    encoding fits only ONE sync-wait command. Spread the excess waits over
    preceding single-wait NoOps on the same (SP) engine."""
    global _DRAIN_PATCHED
    if _DRAIN_PATCHED:
        return
    import concourse.tile as tile_mod

    def _patched(self, tick_clock, wait_clock):
        nc = self.nc
        drain_inst = nc.sync.drain()
        wait_clock.add_sem_waits(
            drain_inst.ins,
            tile_mod.ScopedClock({None: tick_clock.global_clock}),
        )
        mi = drain_inst.ins
        si = mi.sync_info
        waits = list(si.on_wait) if si is not None and si.on_wait else []
        if len(waits) > 1:
            si.on_wait = waits[:1]
            lst = nc.cur_bb.bb.instructions
            assert lst[-1] is mi, "drain is not the last instruction in block"
            drain_obj = lst.pop()
            for w in waits[1:]:
                nop = nc.sync.nop(nofuse=True)
                nsi = nop.ins.sync_info
                if nsi is None:
                    nop.ins.sync_info = type(si)(on_update=[], on_wait=[w])
                else:
                    nsi.on_wait = [w]
            lst.append(drain_obj)
        nc.all_engine_barrier()
        assert self.sems is not None
        popped = nc._tile_sem_poison_stack.pop()
        assert popped is self._sem_poison
        # No in-program semaphore clears / second barrier: the runtime's
        # NEFF epilogue re-zeros the whole semaphore file after every
        # execution, so these ops (~1.4us measured) are redundant.

    tile_mod.TileContext._drain_and_barrier = _patched
    _DRAIN_PATCHED = True


def _split_excess_waits(nc, max_waits: int = 1):
    """This walrus build encodes at most one sync-wait command per
    instruction. Hoist excess waits onto same-engine NoOps inserted
    immediately before the over-subscribed instruction."""
    import concourse.mybir as mybir

    for fn in nc.m.functions:
        for bb in fn.blocks:
            insts = bb.instructions
            i = 0
            while i < len(insts):
                inst = insts[i]
                si = getattr(inst, "sync_info", None)
                waits = list(si.on_wait) if si is not None and si.on_wait else []
                if len(waits) > max_waits:
                    si.on_wait = waits[:max_waits]
                    extra = waits[max_waits:]
                    nops = []
                    for k in range(0, len(extra), max_waits):
                        nops.append(
                            mybir.InstNoOp(
                                name=f"{inst.name}-xw{k}",
                                engine=inst.engine,
                                bass_nofuse=True,
                                sync_info=mybir.SyncInfo(
                                    on_wait=extra[k : k + max_waits], on_update=[]
                                ),
                            )
                        )
                    insts[i:i] = nops
                    i += len(nops)
                i += 1


# --------------------------------------------------------------------------
# host-side preprocessing
# --------------------------------------------------------------------------
def _fit_polys(iw_params: np.ndarray, te_freq: np.ndarray, te_phase: np.ndarray):
    """Least-squares fit of g_c(a) = sum_t iw[t,c] cos(a f_t + p_t), a in [0,1].

    Returns C[k, c] for k=0..DEG (monomial basis, increasing order), float64.
    """
    npts = 1024
    x = 0.5 * (1.0 + np.cos(np.pi * (np.arange(npts) + 0.5) / npts))
    f = te_freq.astype(np.float64)
    p = te_phase.astype(np.float64)
    iw = iw_params.astype(np.float64)
    G = np.cos(x[:, None] * f[None, :] + p[None, :]) @ iw      # (npts, 5)
    V = np.vander(x, DEG + 1, increasing=True)                 # (npts, DEG+1)
    C, *_ = np.linalg.lstsq(V, G, rcond=None)
    return C  # (DEG+1, 5) float64


def _host_g_plane(A, adj, Cpoly):
    """Selected time-encoding plane g_{adj}(A), adj==0 -> NEG_INF. (B,N,N) f32."""
    idx = np.clip(adj - 1, 0, 4)
    Af = A.astype(np.float64)
    g = Cpoly[DEG][idx]
    for k in range(DEG - 1, -1, -1):
        g = g * Af + Cpoly[k][idx]
    valid = (adj >= 1) & (adj <= 5)
    return np.where(valid, g, NEG_INF).astype(np.float32)


# --------------------------------------------------------------------------
# Bass program (input-independent; SPMD across 8 cores)
# --------------------------------------------------------------------------
def _build_program():
    import concourse.bass as bass
    import concourse.mybir as mybir
    import concourse.tile as tile

    _patch_tail_drain()

    f32 = mybir.dt.float32
    f16 = mybir.dt.float16
    bf16 = mybir.dt.bfloat16
    i8 = mybir.dt.int8
    Alu = mybir.AluOpType
    Act = mybir.ActivationFunctionType

    nc = bass.Bass()

    # DRAM I/O (per-core layouts; host arranges)
    hT_d = nc.dram_tensor("hT", [128, DCH * BL * 128], f16, kind="ExternalInput")  # [dl,(ch,b,i)]
    a_d = nc.dram_tensor("ap", [128, DCH * 5], f32, kind="ExternalInput")          # [dl,(ch,c)]
    h_d = nc.dram_tensor("h", [N, FBD], bf16, kind="ExternalInput")                # [node,(b,d)]
    gt_d = nc.dram_tensor("gt", [N, FBJ], f32, kind="ExternalInput")               # [j,(b,i)]
    madj_d = nc.dram_tensor("madj", [N, 4 * FBJ], i8, kind="ExternalInput")        # adj==2..5
    out_d = nc.dram_tensor("out", [N, FBD], bf16, kind="ExternalOutput")           # [i,(b,d)]

    with tile.TileContext(nc) as tc, ExitStack() as ctx:
        io = ctx.enter_context(tc.tile_pool(name="io", bufs=1))
        wrk = ctx.enter_context(tc.tile_pool(name="wrk", bufs=1))

        # ---- loads. hT quarters split over the sync+scalar rings, paired
        # so batches 0/1 (ch0 AND ch1) land first and batch-0 e1 can
        # complete earliest. gt/madj/h ride the gpsimd ring (mid/late use).
        hT_sb = io.tile([128, DCH * BL * 128], f16, tag="hT")
        nc.sync.dma_start(hT_sb[:, 0:256], hT_d[:, 0:256])          # ch0 b01
        nc.scalar.dma_start(hT_sb[:, 512:768], hT_d[:, 512:768])    # ch1 b01
        nc.sync.dma_start(hT_sb[:, 256:512], hT_d[:, 256:512])      # ch0 b23
        nc.scalar.dma_start(hT_sb[:, 768:1024], hT_d[:, 768:1024])  # ch1 b23
        cst = io.tile([128, 512], f16, tag="cst")
        nc.gpsimd.memset(cst[:], 0.0)
        one_sb = io.tile([128, 1], bf16, tag="one")
        nc.gpsimd.memset(one_sb[:], 1.0)
        a_sb = io.tile([128, DCH * 5], f32, tag="ap")
        nc.gpsimd.dma_start(a_sb[:], a_d[:])
        madj_sb = io.tile([N, 4 * FBJ], i8, tag="madj")
        nc.gpsimd.dma_start(madj_sb[:], madj_d[:])
        gt_sb = io.tile([N, FBJ], f32, tag="gt")
        nc.gpsimd.dma_start(gt_sb[:], gt_d[:])
        h_sb = io.tile([N, FBD], bf16, tag="h")
        nc.gpsimd.dma_start(h_sb[:], h_d[:])

        # ---- PSUM layout
        psE = ctx.enter_context(tc.tile_pool(name="psE", bufs=1, space="PSUM"))
        E03 = psE.tile([N, BL * 512], f32, tag="E03", name="E03")   # (b, c0..3, j)
        E4 = psE.tile([N, FBJ], f32, tag="E4", name="E4")           # (b, j)
        zw = psE.tile([N, 512], f32, tag="zw", name="zw")           # z cols 0:4 | warmup 256:288
        psB = ctx.enter_context(tc.tile_pool(name="psB", bufs=2, space="PSUM"))

        sT = wrk.tile([N, FBJ], f32, tag="sT")
        exT = wrk.tile([N, FBJ], bf16, tag="exT")
        rz = wrk.tile([N, BL], f32, tag="rz")
        outT = wrk.tile([N, FBD], bf16, tag="outT")

        # ---- PE warmup: dummy matmuls keep the PE continuously busy until
        # the real matmuls arrive so DVFS ramps to full clock (a cold PE
        # runs matmuls ~3x slower). 512-free each ~0.4-0.8us of busy.
        for w in range(10):
            nc.tensor.matmul(zw[0:32, 0:512], cst[:, 0:32], cst[:, 0:512],
                             skip_group_check=True)

        # ---- hTa = hT * a_c on DVE tensor_scalar (fp16 2x, ~420ns/op),
        # one op per (class, chunk) spanning all 4 batches; the two
        # ch1-tail classes run on the scalar engine concurrently.
        # hTa layout [dl,(ch,b,c,j)] keeps matmul moving operands contiguous.
        hTa = wrk.tile([128, 5 * DCH * BL * 128], f16, tag="hTa")
        hTa_v = hTa[:].rearrange("p (ch b c j) -> p ch c b j",
                                 ch=DCH, b=BL, c=5, j=128)
        hT_v = hT_sb[:].rearrange("p (ch b i) -> p ch b i", ch=DCH, b=BL, i=128)
        for ch in range(DCH):
            for c in range(5):
                if ch == 1 and c >= 3:
                    nc.scalar.mul(hTa_v[:, ch, c], hT_v[:, ch],
                                  a_sb[:, ch * 5 + c : ch * 5 + c + 1])
                else:
                    nc.vector.tensor_scalar(
                        hTa_v[:, ch, c], hT_v[:, ch],
                        a_sb[:, ch * 5 + c : ch * 5 + c + 1], None, Alu.mult)

        # warm the scalar engine's Exp act table (1.3us load) in its
        # post-scale idle window, well before exp; emitted AFTER the scale
        # ops so it doesn't delay them on the in-order scalar queue
        exwarm = wrk.tile([128, 1], bf16, tag="exwarm")
        nc.scalar.activation(exwarm[:], cst[:, 0:1], Act.Exp)

        # ---- e1 matmuls: stationary = unscaled hT chunk shared by all 5
        # classes of a (ch,b); classes 0-3 into one 512-wide bank per batch,
        # class 4 packed (b,j) in a fifth bank. fp16, accumulated over ch.
        for b in range(BL):
            for ch in range(DCH):
                lhsT = hT_sb[:, (ch * BL + b) * 128 : (ch * BL + b + 1) * 128]
                mov = (ch * BL + b) * 640
                nc.tensor.matmul(
                    E4[:, b * 128 : (b + 1) * 128],
                    lhsT,
                    hTa[:, mov + 512 : mov + 640],
                    start=(ch == 0),
                    stop=(ch == DCH - 1),
                )
                nc.tensor.matmul(
                    E03[:, b * 512 : (b + 1) * 512],
                    lhsT,
                    hTa[:, mov : mov + 512],
                    start=(ch == 0),
                    stop=(ch == DCH - 1),
                )

        # ---- class select IN PLACE in PSUM: the class-0 slots of E03 become
        # the selected scores (no base copy needed); 4 full-width predicated
        # overwrites for classes 1..4. adj==0 cells get drowned by GT=-9e15.
        Ev = E03[:].rearrange("p (b c j) -> p c b j", b=BL, c=4, j=128)
        sel = Ev[:, 0]                                   # [p, b, j] strides (512,1)
        for k in range(3):   # classes 1..3
            nc.vector.copy_predicated(
                sel,
                madj_sb[:, k * FBJ : (k + 1) * FBJ].rearrange(
                    "p (b j) -> p b j", b=BL, j=128),
                Ev[:, k + 1],
            )
        nc.vector.copy_predicated(
            sel,
            madj_sb[:, 3 * FBJ : 4 * FBJ].rearrange("p (b j) -> p b j", b=BL, j=128),
            E4[:].rearrange("p (b j) -> p b j", b=BL, j=128),
        )

        # ---- s = lrelu(s + GT), exp -> bf16. Full-width ops: less total
        # DVE stream than batch-pair splits and a shallower path to the
        # last exp, which gates the tail. Keep the PE warm meanwhile.
        for w in range(4):
            nc.tensor.matmul(zw[0:32, 0:512], cst[:, 0:32], cst[:, 0:512],
                             skip_group_check=True)
        sel_all = Ev[:, 0]                               # [p, 4, 128]
        sT_v = sT[:].rearrange("p (b j) -> p b j", b=BL, j=128)
        nc.vector.scalar_tensor_tensor(
            sT_v, sel_all, 1.0,
            gt_sb[:].rearrange("p (b j) -> p b j", b=BL, j=128),
            Alu.mult, Alu.add,
        )
        nc.vector.scalar_tensor_tensor(
            sT[:], sT[:], ALPHA, sT[:], Alu.mult, Alu.max)
        nc.scalar.activation(exT[:], sT[:], Act.Exp)

        # ---- softmax backend: z via ones-matmul sharing the exT stationary
        # with the output matmul; rescale by 1/z on the way out; outputs on
        # two rings as batch pairs.
        ops = []
        for b in range(BL):
            exT_b = exT[:, b * 128 : (b + 1) * 128]
            nc.tensor.matmul(zw[:, b : b + 1], exT_b, one_sb[:, 0:1])
            op = psB.tile([N, D], f32, tag="outp", name=f"outp{b}")
            nc.tensor.matmul(op[:], exT_b, h_sb[:, b * D : (b + 1) * D])
            ops.append(op)
            nc.vector.reciprocal(rz[:, b : b + 1], zw[:, b : b + 1])
            # DVE is idle here (exp went full-width): split the four rescale
            # muls across DVE/scalar so the last store issues ~1us sooner
            if b % 2 == 0:
                nc.vector.tensor_scalar(
                    outT[:, b * D : (b + 1) * D], ops[b][:],
                    rz[:, b : b + 1], None, Alu.mult)
            else:
                nc.scalar.mul(outT[:, b * D : (b + 1) * D], ops[b][:],
                              rz[:, b : b + 1])
            q = nc.sync if b % 2 == 0 else nc.gpsimd
            q.dma_start(out_d[:, b * D : (b + 1) * D],
                        outT[:, b * D : (b + 1) * D])

    return nc


# --------------------------------------------------------------------------
# host packing
# --------------------------------------------------------------------------
def _prepare_in_maps(inputs):
    hidden = np.ascontiguousarray(inputs["hidden"], dtype=np.float32)   # (B,N,D)
    A = np.ascontiguousarray(inputs["A_interval"], dtype=np.float32)    # (B,N,N)
    adj = np.asarray(inputs["adj"])                                     # (B,N,N) i32
    a_params = np.asarray(inputs["a_params"], dtype=np.float32)         # (D,5)
    iw = np.asarray(inputs["iw_params"])
    f = np.asarray(inputs["te_freq"])
    p = np.asarray(inputs["te_phase"])

    Cpoly = _fit_polys(iw, f, p)
    G = _host_g_plane(A, adj, Cpoly)                                    # (B,N,N) f32

    in_maps = []
    for core in range(NCORES):
        bs = slice(core * BL, (core + 1) * BL)
        hs = hidden[bs]                                   # (BL,N,D)
        assert ((adj[bs] >= 1) & (adj[bs] <= 5)).any(axis=2).all(), (
            "row with no valid edge: shift-free softmax unsupported")
        # h: [node, (b,d)] bf16
        h_host = np.ascontiguousarray(hs.transpose(1, 0, 2)).reshape(
            N, FBD).astype(ml_dtypes.bfloat16)
        # hT: [dl, (ch, b, i)] fp16
        hT_host = np.empty((128, DCH * BL * 128), np.float16)
        for ch in range(DCH):
            for b in range(BL):
                hT_host[:, (ch * BL + b) * 128 : (ch * BL + b + 1) * 128] = (
                    hs[b, :, ch * 128 : (ch + 1) * 128].T
                )
        # a_params -> [dl, (ch, c)] fp16
        ap_host = np.empty((128, DCH * 5), np.float32)
        for ch in range(DCH):
            ap_host[:, ch * 5 : (ch + 1) * 5] = a_params[ch * 128 : (ch + 1) * 128, :]
        # transposed planes [j, (b, i)]
        GT_host = np.ascontiguousarray(G[bs].transpose(2, 0, 1)).reshape(N, FBJ)
        adjT = adj[bs].transpose(2, 0, 1).reshape(N, FBJ)
        madj_host = np.empty((N, 4 * FBJ), np.int8)
        for k in range(4):  # classes 1..4  <->  adj == 2..5
            madj_host[:, k * FBJ : (k + 1) * FBJ] = (adjT == k + 2)
        in_maps.append({
            "hT": hT_host, "ap": ap_host, "h": h_host,
            "gt": GT_host, "madj": madj_host,
        })
    return in_maps


def _get_program():
    nc = _PROG_CACHE.get("prog")
    if nc is None:
        nc = _build_program()
        _split_excess_waits(nc)
        _PROG_CACHE["prog"] = nc
    return nc


# --------------------------------------------------------------------------
# public entry point
# --------------------------------------------------------------------------
def kernel(**inputs: np.ndarray) -> np.ndarray:
    nc = _get_program()
    in_maps = _prepare_in_maps(inputs)

    from concourse.bass_utils import run_bass_kernel_spmd

    res = run_bass_kernel_spmd(nc, in_maps, core_ids=list(range(NCORES)))
    out = np.empty((B, N, D), np.float32)
    for core in range(NCORES):
        o = np.asarray(res.results[core]["out"]).astype(np.float32)
        out[core * BL : (core + 1) * BL] = o.reshape(N, BL, D).transpose(1, 0, 2)
    return out


if __name__ == "__main__":
    rng = np.random.default_rng(0)
    demo = {
        "hidden": rng.standard_normal((B, N, D), dtype=np.float32),
        "A_interval": rng.random((B, N, N), dtype=np.float32),
        "adj": rng.integers(0, 6, (B, N, N)).astype(np.int32),
        "interval_unique": rng.integers(0, 100, (B, N)).astype(np.int32),
        "mask_item": rng.integers(0, 2, (B, N)).astype(np.int32),
        "a_params": (rng.standard_normal((D, 5)) / np.sqrt(D)).astype(np.float32),
        "iw_params": rng.standard_normal((TDIM, 5)).astype(np.float32),
        "te_freq": rng.standard_normal(TDIM).astype(np.float32),
        "te_phase": rng.standard_normal(TDIM).astype(np.float32),
    }
    o = kernel(**demo)
    print("kernel output", o.shape, o.dtype, np.abs(o).max())
